# revision 29
# baseline (speedup 1.0000x reference)
"""Trainium2 Bass kernel for the CoAttention DNS/Image module.

Math notes (exact algebraic simplification of the reference):
  scores1[b,r,s] = s_img[b,r] + s_dns[b,s] + b_att1 ; softmax over s.
  The per-row constant s_img[b,r] (and b_att1) cancels in the softmax, so
  a1[b,r,:] == softmax(s_dns[b,:]) for every r. Hence
      att_dns[b,r,:] = softmax(s_dns[b]) @ dns[b]          (same for all r)
  Similarly scores2's softmax over j kills t_dns[b,i] and b_att2, so
      att_img[b,i,:] = softmax(t_img[b]) @ img[b]          (same for all i)
  Therefore W_img1, w_att1[:H], b_att1, W_dns2, w_att2[:H], b_att2 do not
  affect the outputs at all.  The remaining work per batch item:
      s_dns[s] = tanh(dns[b] @ W_dns1.T + b_dns1) @ w_att1[H:]
      t_img[j] = tanh(img[b] @ W_img2.T + b_img2) @ w_att2[H:]
  plus two tiny softmaxes and two weighted sums, each of which produces a
  single H-vector per item that the host broadcasts to all R output rows.

Distribution: pure data parallel over batch (64 items -> 8 items/core on 8
NeuronCores), no collectives.  The HxH projections run on the PE array in
fp8e4(DoubleRow) (weights pre-scaled x64 so they stay in fp8 normal range;
the tanh activation rescales by 1/64); everything downstream of the tanh is
bf16 with fp32 PSUM accumulation.  CPU-sim rel-err ~1.4e-2 for fp8 proj,
~3e-3 all-bf16, inside the 2e-2 gate.  Inputs are host-staged in the exact
SBUF layouts so every input DMA is a contiguous multi-KB-per-partition
transfer, ordered so pair 0's operands land first.
"""

import os
import sys

import numpy as np

try:
    import concourse  # noqa: F401
except ImportError:  # fresh environment: fall back to the repo path
    sys.path.insert(0, "/opt/trn_rl_repo")

import ml_dtypes

BF16 = ml_dtypes.bfloat16
FP8 = ml_dtypes.float8_e4m3

B, S, R, H = 64, 256, 196, 1024
NCORES = 8
BPC = B // NCORES        # batch items per core = 8
PAIRS = BPC // 2         # items are processed in pairs = 4
HC = H // 128            # 8 chunks of the feature dim
ND = 2 * S               # dns pair free width  = 512
NG = 2 * R               # img pair free width  = 392
NGP = 400                # img pair width padded to a 16B multiple (DoubleRow)
WSC = 64.0               # fp8 weight pre-scale

USE_FP8 = os.environ.get("COATT_FP8", "1") == "1"

_CACHE = {}


def _build_program(use_fp8):
    import concourse.bacc as bacc
    import concourse.tile as tile
    from concourse import mybir
    from contextlib import ExitStack

    f32 = mybir.dt.float32
    bf = mybir.dt.bfloat16
    adt = mybir.dt.float8e4 if use_fp8 else bf   # projection operand dtype
    ngp = NGP if use_fp8 else NG
    Act = mybir.ActivationFunctionType
    DR = mybir.MatmulPerfMode.DoubleRow

    nc = bacc.Bacc("TRN2", target_bir_lowering=False, debug=False)

    # Transposed activations, pair-major: dT[pr] is (128, HC*ND) with free
    # layout (hc, j, s); the (j, s) pair axis is the matmul moving dim.
    dT = nc.dram_tensor("dT", (PAIRS, 128, HC * ND), adt, kind="ExternalInput").ap()
    gT = nc.dram_tensor("gT", (PAIRS, 128, HC * ngp), adt, kind="ExternalInput").ap()
    # Natural-layout activations for the attention-weighted sums, split by
    # 128-row chunk: dN0[p, it*H+h] = dns[it, p, h]; dN1 rows 128..255.
    dN0 = nc.dram_tensor("dN0", (128, BPC * H), bf, kind="ExternalInput").ap()
    dN1 = nc.dram_tensor("dN1", (128, BPC * H), bf, kind="ExternalInput").ap()
    gN0 = nc.dram_tensor("gN0", (128, BPC * H), bf, kind="ExternalInput").ap()
    gN1 = nc.dram_tensor("gN1", (R - 128, BPC * H), bf, kind="ExternalInput").ap()
    # Projection weights, oc-major strips: w1t[p, (oc*HC+hc)*128 + c] =
    # W_dns1[oc*128+c, hc*128+p] (pre-scaled by WSC in fp8 mode).
    w1t = nc.dram_tensor("w1t", (128, HC * H), adt, kind="ExternalInput").ap()
    w4t = nc.dram_tensor("w4t", (128, HC * H), adt, kind="ExternalInput").ap()
    bc1 = nc.dram_tensor("bc1", (128, HC), f32, kind="ExternalInput").ap()
    bc4 = nc.dram_tensor("bc4", (128, HC), f32, kind="ExternalInput").ap()
    wd1 = nc.dram_tensor("wd1", (128, HC), bf, kind="ExternalInput").ap()
    wi2 = nc.dram_tensor("wi2", (128, HC), bf, kind="ExternalInput").ap()
    one1 = nc.dram_tensor("one1", (1, 1), bf, kind="ExternalInput").ap()

    # vout[it, 0, :] = att_dns row for item it; vout[it, 1, :] = att_img row.
    vout = nc.dram_tensor("vout", (BPC, 2, 2, 512), f32, kind="ExternalOutput").ap()

    with tile.TileContext(nc) as tc, ExitStack() as ctx:
        consts = ctx.enter_context(tc.tile_pool(name="consts", bufs=1))
        tpool = ctx.enter_context(tc.tile_pool(name="tpool", bufs=6))
        smalls = ctx.enter_context(tc.tile_pool(name="smalls", bufs=3))
        ets = ctx.enter_context(tc.tile_pool(name="ets", bufs=6))
        vsbp = ctx.enter_context(tc.tile_pool(name="vsbp", bufs=3))
        pproj = ctx.enter_context(tc.tile_pool(name="pproj", bufs=5, space="PSUM"))
        psr = ctx.enter_context(tc.tile_pool(name="psr", bufs=1, space="PSUM"))
        pet = ctx.enter_context(tc.tile_pool(name="pet", bufs=1, space="PSUM"))
        pv = ctx.enter_context(tc.tile_pool(name="pv", bufs=1, space="PSUM"))

        # --- SBUF residents ---
        w1_sb = consts.tile([128, HC * H], adt, name="w1_sb")
        w4_sb = consts.tile([128, HC * H], adt, name="w4_sb")
        b1_sb = consts.tile([128, HC], f32, name="b1_sb")
        b4_sb = consts.tile([128, HC], f32, name="b4_sb")
        wd1_sb = consts.tile([128, HC], bf, name="wd1_sb")
        wi2_sb = consts.tile([128, HC], bf, name="wi2_sb")
        one_sb = consts.tile([1, 1], bf, name="one_sb")
        dT_sb = consts.tile([128, PAIRS * HC * ND], adt, name="dT_sb")
        gT_sb = consts.tile([128, PAIRS * HC * ngp], adt, name="gT_sb")
        dN0_sb = consts.tile([128, BPC * H], bf, name="dN0_sb")
        dN1_sb = consts.tile([128, BPC * H], bf, name="dN1_sb")
        gN0_sb = consts.tile([128, BPC * H], bf, name="gN0_sb")
        gN1_sb = consts.tile([128, BPC * H], bf, name="gN1_sb")

        # --- input DMAs, ordered so pair 0 can start ASAP ---
        def load_pair_T(pr):
            nc.sync.dma_start(
                out=dT_sb[:, pr * HC * ND:(pr + 1) * HC * ND], in_=dT[pr])
            nc.sync.dma_start(
                out=gT_sb[:, pr * HC * ngp:(pr + 1) * HC * ngp], in_=gT[pr])

        def load_item_N(it):
            sl = slice(it * H, (it + 1) * H)
            nc.sync.dma_start(out=dN0_sb[:, sl], in_=dN0[:, sl])
            nc.sync.dma_start(out=dN1_sb[:, sl], in_=dN1[:, sl])
            nc.sync.dma_start(out=gN0_sb[:, sl], in_=gN0[:, sl])
            nc.sync.dma_start(out=gN1_sb[0:R - 128, sl], in_=gN1[:, sl])

        # HAM prewarm: dummy matmuls on a zeroed tile keep the PE busy
        # through its 3.4us activity window while the first input DMAs are
        # in flight, so the real matmuls start at full clock.
        warm = tpool.tile([128, 512], bf, tag="warm", name="warm")
        nc.vector.memset(warm, 0.0)
        for wmm in range(14):
            wp = pproj.tile([128, 512], f32, tag="proj", name=f"warmp{wmm}")
            nc.tensor.matmul(wp, lhsT=warm[:, 0:128], rhs=warm,
                             start=True, stop=True)

        # pair 0's dT in hc-pair chunks split across both HWDGE rings so the
        # first matmul can start after ~128KB; the first oc weight strip
        # right behind it.
        hq = HC * ND // 4
        nc.scalar.dma_start(out=dT_sb[:, 0:hq], in_=dT[0][:, 0:hq])
        nc.scalar.dma_start(
            out=w1_sb[:, 0:H], in_=w1t[:, 0:H])
        nc.sync.dma_start(out=dT_sb[:, hq:2 * hq], in_=dT[0][:, hq:2 * hq])
        nc.scalar.dma_start(out=dT_sb[:, 2 * hq:3 * hq], in_=dT[0][:, 2 * hq:3 * hq])
        nc.sync.dma_start(out=dT_sb[:, 3 * hq:4 * hq], in_=dT[0][:, 3 * hq:4 * hq])
        nc.sync.dma_start(out=b1_sb, in_=bc1)
        nc.sync.dma_start(out=wd1_sb, in_=wd1)
        for oc in range(1, HC):
            nc.sync.dma_start(
                out=w1_sb[:, oc * H:(oc + 1) * H], in_=w1t[:, oc * H:(oc + 1) * H])
        nc.sync.dma_start(out=one_sb, in_=one1)
        nc.sync.dma_start(
            out=gT_sb[:, 0:HC * ngp], in_=gT[0])
        for oc in range(HC):
            nc.sync.dma_start(
                out=w4_sb[:, oc * H:(oc + 1) * H], in_=w4t[:, oc * H:(oc + 1) * H])
        nc.sync.dma_start(out=b4_sb, in_=bc4)
        nc.sync.dma_start(out=wi2_sb, in_=wi2)
        load_item_N(0)
        load_item_N(1)
        for pr in range(1, PAIRS):
            load_pair_T(pr)
            load_item_N(2 * pr)
            load_item_N(2 * pr + 1)

        dT4 = dT_sb.rearrange("p (pr hc n) -> p pr hc n", pr=PAIRS, hc=HC)
        gT4 = gT_sb.rearrange("p (pr hc n) -> p pr hc n", pr=PAIRS, hc=HC)
        w1v = w1_sb.rearrange("p (oc hc c) -> p oc hc c", oc=HC, hc=HC)
        w4v = w4_sb.rearrange("p (oc hc c) -> p oc hc c", oc=HC, hc=HC)

        def make_tail(pr, side, srow, ns, nat0, nat1, r1):
            """Softmax + e-transpose + col-tiled weighted sums + store for
            one (pair, side).  Emitted deferred, in the middle of the NEXT
            side's projection loop, so the PE queue never head-of-line
            blocks on the scalar/DVE softmax chain."""
            def tail():
                eT = []
                rvp = smalls.tile([128, 1], f32, tag="rvp", name=f"rvp{pr}_{side}")
                for j in (0, 1):
                    eb = smalls.tile([1, ns], bf, tag="e", name=f"e{pr}_{side}_{j}")
                    sm = smalls.tile([1, 1], f32, tag="sm", name=f"sm{pr}_{side}_{j}")
                    nc.scalar.activation(
                        out=eb, in_=srow[0:1, j * ns:(j + 1) * ns],
                        func=Act.Exp, accum_out=sm,
                    )
                    rv = smalls.tile([1, 1], f32, tag="rv", name=f"rv{pr}_{side}_{j}")
                    nc.vector.reciprocal(out=rv, in_=sm)
                    # stage 1/d at the partitions holding item j's v rows
                    for k in (2 * j, 2 * j + 1):
                        nc.vector.tensor_copy(
                            out=rvp[32 * k:32 * k + 1, :], in_=rv)

                    # transpose e to the partition axis (eT = e^T via PE)
                    ej = []
                    for sc in (0, 1):
                        w = 128 if sc == 0 else ns - 128
                        etp = pet.tile([128, 1], f32, tag="etp",
                                       name=f"etp{pr}_{side}_{j}_{sc}")
                        nc.tensor.matmul(
                            etp[0:w, :],
                            lhsT=eb[0:1, sc * 128: sc * 128 + w],
                            rhs=one_sb,
                            start=True, stop=True)
                        es = ets.tile([128, 1], bf, tag="eT",
                                      name=f"es{pr}_{side}_{j}_{sc}")
                        nc.vector.tensor_copy(out=es[0:w, :], in_=etp[0:w, :])
                        ej.append(es)
                    eT.append(ej)

                # All four (j, hh) weighted sums run as M=1 matmuls in
                # distinct PE column groups (tile_position), so the two
                # accumulation waves stream concurrently.
                combos = [(j, hh) for j in (0, 1) for hh in (0, 1)]
                vp = pv.tile([128, 512], f32, tag="vp", name=f"vp{pr}_{side}")
                for sc, nat, rows in ((0, nat0, 128), (1, nat1, r1)):
                    for k, (j, hh) in enumerate(combos):
                        it = 2 * pr + j
                        nc.tensor.matmul(
                            vp[32 * k:32 * k + 1, :],
                            lhsT=eT[j][sc][0:rows, :],
                            rhs=nat[0:rows, it * H + hh * 512:
                                    it * H + (hh + 1) * 512],
                            start=(sc == 0), stop=(sc == 1),
                            tile_position=(0, 32 * k))
                vsb = vsbp.tile([128, 512], f32, tag="v", name=f"v{pr}_{side}")
                nc.vector.tensor_scalar_mul(vsb, vp, rvp)
                v4 = vsb.rearrange("(a b) n -> a b n", a=4, b=32)
                for j in (0, 1):
                    it = 2 * pr + j
                    nc.sync.dma_start(
                        out=vout[it, side], in_=v4[2 * j:2 * j + 2, 0, :])
            return tail

        pending_tail = None
        for pr in range(PAIRS):
            for side in (0, 1):
                if side == 0:
                    actv, wv, b_sb, wvec_sb, n, ns = dT4, w1v, b1_sb, wd1_sb, ND, S
                    nat0, nat1, r1 = dN0_sb, dN1_sb, 128
                else:
                    actv, wv, b_sb, wvec_sb, n, ns = gT4, w4v, b4_sb, wi2_sb, NG, R
                    nat0, nat1, r1 = gN0_sb, gN1_sb, R - 128

                # srow[j*ns+s] = sum_o w[o] * tanh(proj[o, j*ns+s] + b[o])
                srow = psr.tile([1, n], f32, tag="srow", name=f"srow{pr}_{side}")
                for oc in range(HC):
                    pj = pproj.tile([128, n], f32, tag="proj",
                                    name=f"pj{pr}_{side}_{oc}")
                    if use_fp8:
                        for hc in range(0, HC, 2):
                            nc.tensor.matmul(
                                pj,
                                lhsT=wv[:, oc, hc:hc + 2, :],
                                rhs=actv[:, pr, hc:hc + 2, 0:n],
                                start=(hc == 0),
                                stop=(hc == HC - 2),
                                perf_mode=DR,
                            )
                    else:
                        for hc in range(HC):
                            nc.tensor.matmul(
                                pj,
                                lhsT=wv[:, oc, hc, :],
                                rhs=actv[:, pr, hc, 0:n],
                                start=(hc == 0),
                                stop=(hc == HC - 1),
                            )
                    tt = tpool.tile([128, n], bf, tag="T", name=f"tt{pr}_{side}_{oc}")
                    nc.scalar.activation(
                        out=tt, in_=pj, func=Act.Tanh,
                        bias=b_sb[:, oc:oc + 1],
                        scale=(1.0 / WSC) if use_fp8 else 1.0,
                    )
                    nc.tensor.matmul(
                        srow,
                        lhsT=wvec_sb[:, oc:oc + 1],
                        rhs=tt,
                        start=(oc == 0),
                        stop=(oc == HC - 1),
                    )
                make_tail(pr, side, srow, ns, nat0, nat1, r1)()
        del pending_tail

    nc.compile()
    return nc


def _get_program():
    key = ("prog", USE_FP8)
    if key not in _CACHE:
        _CACHE[key] = _build_program(USE_FP8)
    return _CACHE[key]


def _stage_core(dns_f, img_f, adt, ngp):
    """Host-side staging of one core's activations into SBUF layouts."""
    dns_bf = np.asarray(dns_f, BF16)
    img_bf = np.asarray(img_f, BF16)
    # transposed, pair-major: (PAIRS, 128, HC*2*S) free layout (hc, j, s)
    x = np.asarray(dns_f, adt).reshape(PAIRS, 2, S, HC, 128)
    dTc = np.ascontiguousarray(x.transpose(0, 4, 3, 1, 2)).reshape(
        PAIRS, 128, HC * ND)
    x = np.asarray(img_f, adt).reshape(PAIRS, 2, R, HC, 128)
    gTc = np.zeros((PAIRS, 128, HC, ngp), adt)
    gTc[:, :, :, 0:NG] = x.transpose(0, 4, 3, 1, 2).reshape(PAIRS, 128, HC, NG)
    gTc = gTc.reshape(PAIRS, 128, HC * ngp)
    # natural row chunks: (128, BPC*H)
    dN0c = np.ascontiguousarray(dns_bf[:, 0:128].transpose(1, 0, 2)).reshape(
        128, BPC * H)
    dN1c = np.ascontiguousarray(dns_bf[:, 128:256].transpose(1, 0, 2)).reshape(
        128, BPC * H)
    gN0c = np.ascontiguousarray(img_bf[:, 0:128].transpose(1, 0, 2)).reshape(
        128, BPC * H)
    gN1c = np.ascontiguousarray(img_bf[:, 128:R].transpose(1, 0, 2)).reshape(
        R - 128, BPC * H)
    return dTc, gTc, dN0c, dN1c, gN0c, gN1c


def _stage_weight(Wmat, adt, scale):
    """(H, H) weight -> (128, HC*H), strips (oc, hc, c):
    w[p, (oc*HC+hc)*128+c] = scale * W[oc*128+c, hc*128+p]."""
    wt = (np.asarray(Wmat, np.float32) * scale).T.reshape(HC, 128, HC, 128)
    return np.ascontiguousarray(
        wt.transpose(1, 2, 0, 3), dtype=adt).reshape(128, HC * H)


def _prepare_in_maps(dns_feature, img_features, W_dns1, b_dns1, W_img2, b_img2,
                     w_att1, w_att2):
    adt = FP8 if USE_FP8 else BF16
    ngp = NGP if USE_FP8 else NG
    wsc = WSC if USE_FP8 else 1.0
    w1t = _stage_weight(W_dns1, adt, wsc)
    w4t = _stage_weight(W_img2, adt, wsc)
    bc1 = np.ascontiguousarray(np.asarray(b_dns1, np.float32).reshape(HC, 128).T)
    bc4 = np.ascontiguousarray(np.asarray(b_img2, np.float32).reshape(HC, 128).T)
    wd1 = np.ascontiguousarray(np.asarray(w_att1, BF16)[H:].reshape(HC, 128).T)
    wi2 = np.ascontiguousarray(np.asarray(w_att2, BF16)[H:].reshape(HC, 128).T)
    one1 = np.ones((1, 1), dtype=BF16)
    dns_f = np.asarray(dns_feature, np.float32)
    img_f = np.asarray(img_features, np.float32)
    in_maps = []
    for c in range(NCORES):
        dTc, gTc, dN0c, dN1c, gN0c, gN1c = _stage_core(
            dns_f[c * BPC:(c + 1) * BPC], img_f[c * BPC:(c + 1) * BPC], adt, ngp)
        in_maps.append({
            "dT": dTc, "gT": gTc,
            "dN0": dN0c, "dN1": dN1c, "gN0": gN0c, "gN1": gN1c,
            "w1t": w1t, "w4t": w4t, "bc1": bc1, "bc4": bc4,
            "wd1": wd1, "wi2": wi2, "one1": one1,
        })
    return in_maps


def run(inputs, trace=False):
    """Run on the 8 NeuronCores; returns (att_img, att_dns, exec_time_ns)."""
    from concourse.bass_utils import run_bass_kernel_spmd

    nc = _get_program()
    in_maps = _prepare_in_maps(
        inputs["dns_feature"], inputs["img_features"],
        inputs["W_dns1"], inputs["b_dns1"], inputs["W_img2"], inputs["b_img2"],
        inputs["w_att1"], inputs["w_att2"],
    )
    res = run_bass_kernel_spmd(nc, in_maps, core_ids=list(range(NCORES)),
                               trace=trace)
    v = np.concatenate([res.results[c]["vout"] for c in range(NCORES)],
                       0).reshape(B, 2, H)
    att_dns = np.broadcast_to(
        np.ascontiguousarray(v[:, 0, :])[:, None, :], (B, R, H))
    att_img = np.broadcast_to(
        np.ascontiguousarray(v[:, 1, :])[:, None, :], (B, R, H))
    return att_img, att_dns, res.exec_time_ns


def kernel(**inputs):
    att_img, att_dns, _ = run(inputs, trace=False)
    return att_img, att_dns


if __name__ == "__main__":
    prog = _get_program()
    print("program built + compiled OK")


# revision 30
# speedup vs baseline: 1.0190x; 1.0190x over previous
"""Trainium2 Bass kernel for the CoAttention DNS/Image module.

Math notes (exact algebraic simplification of the reference):
  scores1[b,r,s] = s_img[b,r] + s_dns[b,s] + b_att1 ; softmax over s.
  The per-row constant s_img[b,r] (and b_att1) cancels in the softmax, so
  a1[b,r,:] == softmax(s_dns[b,:]) for every r. Hence
      att_dns[b,r,:] = softmax(s_dns[b]) @ dns[b]          (same for all r)
  Similarly scores2's softmax over j kills t_dns[b,i] and b_att2, so
      att_img[b,i,:] = softmax(t_img[b]) @ img[b]          (same for all i)
  Therefore W_img1, w_att1[:H], b_att1, W_dns2, w_att2[:H], b_att2 do not
  affect the outputs at all.  The remaining work per batch item:
      s_dns[s] = tanh(dns[b] @ W_dns1.T + b_dns1) @ w_att1[H:]
      t_img[j] = tanh(img[b] @ W_img2.T + b_img2) @ w_att2[H:]
  plus two tiny softmaxes and two weighted sums, each of which produces a
  single H-vector per item that the host broadcasts to all R output rows.

Distribution: pure data parallel over batch (64 items -> 8 items/core on 8
NeuronCores), no collectives.  The HxH projections run on the PE array in
fp8e4(DoubleRow) (weights pre-scaled x64 so they stay in fp8 normal range;
the tanh activation rescales by 1/64); everything downstream of the tanh is
bf16 with fp32 PSUM accumulation.  CPU-sim rel-err ~1.4e-2 for fp8 proj,
~3e-3 all-bf16, inside the 2e-2 gate.  Inputs are host-staged in the exact
SBUF layouts so every input DMA is a contiguous multi-KB-per-partition
transfer, ordered so pair 0's operands land first.
"""

import os
import sys

import numpy as np

try:
    import concourse  # noqa: F401
except ImportError:  # fresh environment: fall back to the repo path
    sys.path.insert(0, "/opt/trn_rl_repo")

import ml_dtypes

BF16 = ml_dtypes.bfloat16
FP8 = ml_dtypes.float8_e4m3

B, S, R, H = 64, 256, 196, 1024
NCORES = 8
BPC = B // NCORES        # batch items per core = 8
PAIRS = BPC // 2         # items are processed in pairs = 4
HC = H // 128            # 8 chunks of the feature dim
ND = 2 * S               # dns pair free width  = 512
NG = 2 * R               # img pair free width  = 392
NGP = 400                # img pair width padded to a 16B multiple (DoubleRow)
WSC = 64.0               # fp8 weight pre-scale

USE_FP8 = os.environ.get("COATT_FP8", "1") == "1"

_CACHE = {}


def _build_program(use_fp8):
    import concourse.bacc as bacc
    import concourse.tile as tile
    from concourse import mybir
    from contextlib import ExitStack

    f32 = mybir.dt.float32
    bf = mybir.dt.bfloat16
    adt = mybir.dt.float8e4 if use_fp8 else bf   # projection operand dtype
    ngp = NGP if use_fp8 else NG
    Act = mybir.ActivationFunctionType
    DR = mybir.MatmulPerfMode.DoubleRow

    nc = bacc.Bacc("TRN2", target_bir_lowering=False, debug=False)

    # Transposed activations, pair-major: dT[pr] is (128, HC*ND) with free
    # layout (hc, j, s); the (j, s) pair axis is the matmul moving dim.
    dT = nc.dram_tensor("dT", (PAIRS, 128, HC * ND), adt, kind="ExternalInput").ap()
    gT = nc.dram_tensor("gT", (PAIRS, 128, HC * ngp), adt, kind="ExternalInput").ap()
    # Natural-layout activations for the attention-weighted sums, split by
    # 128-row chunk: dN0[p, it*H+h] = dns[it, p, h]; dN1 rows 128..255.
    dN0 = nc.dram_tensor("dN0", (128, BPC * H), bf, kind="ExternalInput").ap()
    dN1 = nc.dram_tensor("dN1", (128, BPC * H), bf, kind="ExternalInput").ap()
    gN0 = nc.dram_tensor("gN0", (128, BPC * H), bf, kind="ExternalInput").ap()
    gN1 = nc.dram_tensor("gN1", (R - 128, BPC * H), bf, kind="ExternalInput").ap()
    # Projection weights, oc-major strips: w1t[p, (oc*HC+hc)*128 + c] =
    # W_dns1[oc*128+c, hc*128+p] (pre-scaled by WSC in fp8 mode).
    w1t = nc.dram_tensor("w1t", (128, HC * H), adt, kind="ExternalInput").ap()
    w4t = nc.dram_tensor("w4t", (128, HC * H), adt, kind="ExternalInput").ap()
    bc1 = nc.dram_tensor("bc1", (128, HC), f32, kind="ExternalInput").ap()
    bc4 = nc.dram_tensor("bc4", (128, HC), f32, kind="ExternalInput").ap()
    wd1 = nc.dram_tensor("wd1", (128, HC), bf, kind="ExternalInput").ap()
    wi2 = nc.dram_tensor("wi2", (128, HC), bf, kind="ExternalInput").ap()
    one1 = nc.dram_tensor("one1", (1, 1), bf, kind="ExternalInput").ap()

    # vout[it, 0, :] = att_dns row for item it; vout[it, 1, :] = att_img row.
    vout = nc.dram_tensor("vout", (BPC, 2, 2, 512), f32, kind="ExternalOutput").ap()

    with tile.TileContext(nc) as tc, ExitStack() as ctx:
        consts = ctx.enter_context(tc.tile_pool(name="consts", bufs=1))
        tpool = ctx.enter_context(tc.tile_pool(name="tpool", bufs=6))
        smalls = ctx.enter_context(tc.tile_pool(name="smalls", bufs=3))
        ets = ctx.enter_context(tc.tile_pool(name="ets", bufs=6))
        vsbp = ctx.enter_context(tc.tile_pool(name="vsbp", bufs=3))
        pproj = ctx.enter_context(tc.tile_pool(name="pproj", bufs=4, space="PSUM"))
        psr = ctx.enter_context(tc.tile_pool(name="psr", bufs=1, space="PSUM"))
        pet = ctx.enter_context(tc.tile_pool(name="pet", bufs=1, space="PSUM"))
        pv = ctx.enter_context(tc.tile_pool(name="pv", bufs=2, space="PSUM"))

        # --- SBUF residents ---
        w1_sb = consts.tile([128, HC * H], adt, name="w1_sb")
        w4_sb = consts.tile([128, HC * H], adt, name="w4_sb")
        b1_sb = consts.tile([128, HC], f32, name="b1_sb")
        b4_sb = consts.tile([128, HC], f32, name="b4_sb")
        wd1_sb = consts.tile([128, HC], bf, name="wd1_sb")
        wi2_sb = consts.tile([128, HC], bf, name="wi2_sb")
        one_sb = consts.tile([1, 1], bf, name="one_sb")
        dT_sb = consts.tile([128, PAIRS * HC * ND], adt, name="dT_sb")
        gT_sb = consts.tile([128, PAIRS * HC * ngp], adt, name="gT_sb")
        dN0_sb = consts.tile([128, BPC * H], bf, name="dN0_sb")
        dN1_sb = consts.tile([128, BPC * H], bf, name="dN1_sb")
        gN0_sb = consts.tile([128, BPC * H], bf, name="gN0_sb")
        gN1_sb = consts.tile([128, BPC * H], bf, name="gN1_sb")

        # --- input DMAs, ordered so pair 0 can start ASAP ---
        def load_pair_T(pr):
            nc.sync.dma_start(
                out=dT_sb[:, pr * HC * ND:(pr + 1) * HC * ND], in_=dT[pr])
            nc.sync.dma_start(
                out=gT_sb[:, pr * HC * ngp:(pr + 1) * HC * ngp], in_=gT[pr])

        def load_item_N(it):
            sl = slice(it * H, (it + 1) * H)
            nc.sync.dma_start(out=dN0_sb[:, sl], in_=dN0[:, sl])
            nc.sync.dma_start(out=dN1_sb[:, sl], in_=dN1[:, sl])
            nc.sync.dma_start(out=gN0_sb[:, sl], in_=gN0[:, sl])
            nc.sync.dma_start(out=gN1_sb[0:R - 128, sl], in_=gN1[:, sl])

        # HAM prewarm: dummy matmuls on a zeroed tile keep the PE busy
        # through its 3.4us activity window while the first input DMAs are
        # in flight, so the real matmuls start at full clock.
        warm = tpool.tile([128, 512], bf, tag="warm", name="warm")
        nc.vector.memset(warm, 0.0)
        for wmm in range(14):
            wp = pproj.tile([128, 512], f32, tag="proj", name=f"warmp{wmm}")
            nc.tensor.matmul(wp, lhsT=warm[:, 0:128], rhs=warm,
                             start=True, stop=True)

        # pair 0's dT in hc-pair chunks split across both HWDGE rings so the
        # first matmul can start after ~128KB; the first oc weight strip
        # right behind it.
        hq = HC * ND // 4
        nc.scalar.dma_start(out=dT_sb[:, 0:hq], in_=dT[0][:, 0:hq])
        nc.scalar.dma_start(
            out=w1_sb[:, 0:H], in_=w1t[:, 0:H])
        nc.sync.dma_start(out=dT_sb[:, hq:2 * hq], in_=dT[0][:, hq:2 * hq])
        nc.scalar.dma_start(out=dT_sb[:, 2 * hq:3 * hq], in_=dT[0][:, 2 * hq:3 * hq])
        nc.sync.dma_start(out=dT_sb[:, 3 * hq:4 * hq], in_=dT[0][:, 3 * hq:4 * hq])
        nc.sync.dma_start(out=b1_sb, in_=bc1)
        nc.sync.dma_start(out=wd1_sb, in_=wd1)
        for oc in range(1, HC):
            nc.sync.dma_start(
                out=w1_sb[:, oc * H:(oc + 1) * H], in_=w1t[:, oc * H:(oc + 1) * H])
        nc.sync.dma_start(out=one_sb, in_=one1)
        nc.sync.dma_start(
            out=gT_sb[:, 0:HC * ngp], in_=gT[0])
        for oc in range(HC):
            nc.sync.dma_start(
                out=w4_sb[:, oc * H:(oc + 1) * H], in_=w4t[:, oc * H:(oc + 1) * H])
        nc.sync.dma_start(out=b4_sb, in_=bc4)
        nc.sync.dma_start(out=wi2_sb, in_=wi2)
        load_item_N(0)
        load_item_N(1)
        for pr in range(1, PAIRS):
            load_pair_T(pr)
            load_item_N(2 * pr)
            load_item_N(2 * pr + 1)

        dT4 = dT_sb.rearrange("p (pr hc n) -> p pr hc n", pr=PAIRS, hc=HC)
        gT4 = gT_sb.rearrange("p (pr hc n) -> p pr hc n", pr=PAIRS, hc=HC)
        w1v = w1_sb.rearrange("p (oc hc c) -> p oc hc c", oc=HC, hc=HC)
        w4v = w4_sb.rearrange("p (oc hc c) -> p oc hc c", oc=HC, hc=HC)

        def make_tail(pr, side, srow, ns, nat0, nat1, r1):
            """Softmax + e-transpose + col-tiled weighted sums + store for
            one (pair, side).  Emitted deferred, in the middle of the NEXT
            side's projection loop, so the PE queue never head-of-line
            blocks on the scalar/DVE softmax chain."""
            def tail():
                eT = []
                rvp = smalls.tile([128, 1], f32, tag="rvp", name=f"rvp{pr}_{side}")
                for j in (0, 1):
                    eb = smalls.tile([1, ns], bf, tag="e", name=f"e{pr}_{side}_{j}")
                    sm = smalls.tile([1, 1], f32, tag="sm", name=f"sm{pr}_{side}_{j}")
                    nc.scalar.activation(
                        out=eb, in_=srow[0:1, j * ns:(j + 1) * ns],
                        func=Act.Exp, accum_out=sm,
                    )
                    rv = smalls.tile([1, 1], f32, tag="rv", name=f"rv{pr}_{side}_{j}")
                    nc.vector.reciprocal(out=rv, in_=sm)
                    # stage 1/d at the partitions holding item j's v rows
                    for k in (2 * j, 2 * j + 1):
                        nc.vector.tensor_copy(
                            out=rvp[32 * k:32 * k + 1, :], in_=rv)

                    # transpose e to the partition axis (eT = e^T via PE)
                    ej = []
                    for sc in (0, 1):
                        w = 128 if sc == 0 else ns - 128
                        etp = pet.tile([128, 1], f32, tag="etp",
                                       name=f"etp{pr}_{side}_{j}_{sc}")
                        nc.tensor.matmul(
                            etp[0:w, :],
                            lhsT=eb[0:1, sc * 128: sc * 128 + w],
                            rhs=one_sb,
                            start=True, stop=True)
                        es = ets.tile([128, 1], bf, tag="eT",
                                      name=f"es{pr}_{side}_{j}_{sc}")
                        nc.vector.tensor_copy(out=es[0:w, :], in_=etp[0:w, :])
                        ej.append(es)
                    eT.append(ej)

                # All four (j, hh) weighted sums run as M=1 matmuls in
                # distinct PE column groups (tile_position), so the two
                # accumulation waves stream concurrently.
                combos = [(j, hh) for j in (0, 1) for hh in (0, 1)]
                vp = pv.tile([128, 512], f32, tag="vp", name=f"vp{pr}_{side}")
                for sc, nat, rows in ((0, nat0, 128), (1, nat1, r1)):
                    for k, (j, hh) in enumerate(combos):
                        it = 2 * pr + j
                        nc.tensor.matmul(
                            vp[32 * k:32 * k + 1, :],
                            lhsT=eT[j][sc][0:rows, :],
                            rhs=nat[0:rows, it * H + hh * 512:
                                    it * H + (hh + 1) * 512],
                            start=(sc == 0), stop=(sc == 1),
                            tile_position=(0, 32 * k))
                vsb = vsbp.tile([128, 512], f32, tag="v", name=f"v{pr}_{side}")
                nc.vector.tensor_scalar_mul(vsb, vp, rvp)
                v4 = vsb.rearrange("(a b) n -> a b n", a=4, b=32)
                for j in (0, 1):
                    it = 2 * pr + j
                    nc.sync.dma_start(
                        out=vout[it, side], in_=v4[2 * j:2 * j + 2, 0, :])
            return tail

        pending_tail = None
        for pr in range(PAIRS):
            for side in (0, 1):
                if side == 0:
                    actv, wv, b_sb, wvec_sb, n, ns = dT4, w1v, b1_sb, wd1_sb, ND, S
                    nat0, nat1, r1 = dN0_sb, dN1_sb, 128
                else:
                    actv, wv, b_sb, wvec_sb, n, ns = gT4, w4v, b4_sb, wi2_sb, NG, R
                    nat0, nat1, r1 = gN0_sb, gN1_sb, R - 128

                # srow[j*ns+s] = sum_o w[o] * tanh(proj[o, j*ns+s] + b[o])
                srow = psr.tile([1, n], f32, tag="srow", name=f"srow{pr}_{side}")
                for oc in range(HC):
                    pj = pproj.tile([128, n], f32, tag="proj",
                                    name=f"pj{pr}_{side}_{oc}")
                    if use_fp8:
                        for hc in range(0, HC, 2):
                            nc.tensor.matmul(
                                pj,
                                lhsT=wv[:, oc, hc:hc + 2, :],
                                rhs=actv[:, pr, hc:hc + 2, 0:n],
                                start=(hc == 0),
                                stop=(hc == HC - 2),
                                perf_mode=DR,
                            )
                    else:
                        for hc in range(HC):
                            nc.tensor.matmul(
                                pj,
                                lhsT=wv[:, oc, hc, :],
                                rhs=actv[:, pr, hc, 0:n],
                                start=(hc == 0),
                                stop=(hc == HC - 1),
                            )
                    tt = tpool.tile([128, n], bf, tag="T", name=f"tt{pr}_{side}_{oc}")
                    nc.scalar.activation(
                        out=tt, in_=pj, func=Act.Tanh,
                        bias=b_sb[:, oc:oc + 1],
                        scale=(1.0 / WSC) if use_fp8 else 1.0,
                    )
                    nc.tensor.matmul(
                        srow,
                        lhsT=wvec_sb[:, oc:oc + 1],
                        rhs=tt,
                        start=(oc == 0),
                        stop=(oc == HC - 1),
                    )
                make_tail(pr, side, srow, ns, nat0, nat1, r1)()
        del pending_tail

    nc.compile()
    return nc


def _get_program():
    key = ("prog", USE_FP8)
    if key not in _CACHE:
        _CACHE[key] = _build_program(USE_FP8)
    return _CACHE[key]


def _stage_core(dns_f, img_f, adt, ngp):
    """Host-side staging of one core's activations into SBUF layouts."""
    dns_bf = np.asarray(dns_f, BF16)
    img_bf = np.asarray(img_f, BF16)
    # transposed, pair-major: (PAIRS, 128, HC*2*S) free layout (hc, j, s)
    x = np.asarray(dns_f, adt).reshape(PAIRS, 2, S, HC, 128)
    dTc = np.ascontiguousarray(x.transpose(0, 4, 3, 1, 2)).reshape(
        PAIRS, 128, HC * ND)
    x = np.asarray(img_f, adt).reshape(PAIRS, 2, R, HC, 128)
    gTc = np.zeros((PAIRS, 128, HC, ngp), adt)
    gTc[:, :, :, 0:NG] = x.transpose(0, 4, 3, 1, 2).reshape(PAIRS, 128, HC, NG)
    gTc = gTc.reshape(PAIRS, 128, HC * ngp)
    # natural row chunks: (128, BPC*H)
    dN0c = np.ascontiguousarray(dns_bf[:, 0:128].transpose(1, 0, 2)).reshape(
        128, BPC * H)
    dN1c = np.ascontiguousarray(dns_bf[:, 128:256].transpose(1, 0, 2)).reshape(
        128, BPC * H)
    gN0c = np.ascontiguousarray(img_bf[:, 0:128].transpose(1, 0, 2)).reshape(
        128, BPC * H)
    gN1c = np.ascontiguousarray(img_bf[:, 128:R].transpose(1, 0, 2)).reshape(
        R - 128, BPC * H)
    return dTc, gTc, dN0c, dN1c, gN0c, gN1c


def _stage_weight(Wmat, adt, scale):
    """(H, H) weight -> (128, HC*H), strips (oc, hc, c):
    w[p, (oc*HC+hc)*128+c] = scale * W[oc*128+c, hc*128+p]."""
    wt = (np.asarray(Wmat, np.float32) * scale).T.reshape(HC, 128, HC, 128)
    return np.ascontiguousarray(
        wt.transpose(1, 2, 0, 3), dtype=adt).reshape(128, HC * H)


def _prepare_in_maps(dns_feature, img_features, W_dns1, b_dns1, W_img2, b_img2,
                     w_att1, w_att2):
    adt = FP8 if USE_FP8 else BF16
    ngp = NGP if USE_FP8 else NG
    wsc = WSC if USE_FP8 else 1.0
    w1t = _stage_weight(W_dns1, adt, wsc)
    w4t = _stage_weight(W_img2, adt, wsc)
    bc1 = np.ascontiguousarray(np.asarray(b_dns1, np.float32).reshape(HC, 128).T)
    bc4 = np.ascontiguousarray(np.asarray(b_img2, np.float32).reshape(HC, 128).T)
    wd1 = np.ascontiguousarray(np.asarray(w_att1, BF16)[H:].reshape(HC, 128).T)
    wi2 = np.ascontiguousarray(np.asarray(w_att2, BF16)[H:].reshape(HC, 128).T)
    one1 = np.ones((1, 1), dtype=BF16)
    dns_f = np.asarray(dns_feature, np.float32)
    img_f = np.asarray(img_features, np.float32)
    in_maps = []
    for c in range(NCORES):
        dTc, gTc, dN0c, dN1c, gN0c, gN1c = _stage_core(
            dns_f[c * BPC:(c + 1) * BPC], img_f[c * BPC:(c + 1) * BPC], adt, ngp)
        in_maps.append({
            "dT": dTc, "gT": gTc,
            "dN0": dN0c, "dN1": dN1c, "gN0": gN0c, "gN1": gN1c,
            "w1t": w1t, "w4t": w4t, "bc1": bc1, "bc4": bc4,
            "wd1": wd1, "wi2": wi2, "one1": one1,
        })
    return in_maps


def run(inputs, trace=False):
    """Run on the 8 NeuronCores; returns (att_img, att_dns, exec_time_ns)."""
    from concourse.bass_utils import run_bass_kernel_spmd

    nc = _get_program()
    in_maps = _prepare_in_maps(
        inputs["dns_feature"], inputs["img_features"],
        inputs["W_dns1"], inputs["b_dns1"], inputs["W_img2"], inputs["b_img2"],
        inputs["w_att1"], inputs["w_att2"],
    )
    res = run_bass_kernel_spmd(nc, in_maps, core_ids=list(range(NCORES)),
                               trace=trace)
    v = np.concatenate([res.results[c]["vout"] for c in range(NCORES)],
                       0).reshape(B, 2, H)
    att_dns = np.broadcast_to(
        np.ascontiguousarray(v[:, 0, :])[:, None, :], (B, R, H))
    att_img = np.broadcast_to(
        np.ascontiguousarray(v[:, 1, :])[:, None, :], (B, R, H))
    return att_img, att_dns, res.exec_time_ns


def kernel(**inputs):
    att_img, att_dns, _ = run(inputs, trace=False)
    return att_img, att_dns


if __name__ == "__main__":
    prog = _get_program()
    print("program built + compiled OK")
